# revision 1
# baseline (speedup 1.0000x reference)
"""AttentionBlock (GroupNorm + single-head NxN attention + residual) on 8 TRN2 cores.

Data-parallel: batch dim (B=8) sharded 1 batch-image per NeuronCore. Each core
runs the full block for its image:

  x (C=256, N=4096) -> GroupNorm(8 groups) -> qkv = W_qkv @ xn  ->
  sT = k^T q (scores, keys on partitions), e = exp(sT/16)       ->
  den[n] = sum_m e[m,n] (ones-matmul), attnout_u = v @ e        ->
  proj_u = W_out @ attnout_u; out = proj_u * (1/den) + b_out + x

All heavy matmuls run as float32r (full-rate PE, fp32 storage). The softmax is
computed unnormalized; the 1/den scale commutes through the output projection
and is applied once at the end (per-column broadcast via DMA).
"""

import sys

if "/opt/trn_rl_repo" not in sys.path:
    sys.path.insert(0, "/opt/trn_rl_repo")

import numpy as np

import concourse.bass as bass
import concourse.bacc as bacc
import concourse.tile as tile
import concourse.mybir as mybir
from concourse import bass_utils

# Problem dims (hardcoded per spec)
B, C, HH, WW = 8, 256, 64, 64
N = HH * WW            # 4096
G = 8                  # groupnorm groups
GSZ = C // G           # 32 channels/group
EPS = 1e-5
P = 128                # SBUF partitions
CT = C // P            # 2 channel tiles
NCH = 512              # query-chunk width (free dim per matmul)
NNCH = N // NCH        # 8
MT = N // P            # 32 key tiles
SCALE = 1.0 / np.sqrt(C)
INV_CNT = 1.0 / (GSZ * N)

F32 = mybir.dt.float32
F32R = mybir.dt.float32r


def _emit(tc, d, out_d):
    from contextlib import ExitStack

    nc = tc.nc
    AF = mybir.ActivationFunctionType
    OP = mybir.AluOpType
    AX = mybir.AxisListType.X
    ts, ds = bass.ts, bass.ds

    with ExitStack() as ctx:
        const = ctx.enter_context(tc.tile_pool(name="const", bufs=1))
        big = ctx.enter_context(tc.tile_pool(name="big", bufs=1))
        work = ctx.enter_context(tc.tile_pool(name="work", bufs=3))
        small = ctx.enter_context(tc.tile_pool(name="small", bufs=4))
        outp = ctx.enter_context(tc.tile_pool(name="outp", bufs=3))
        psS = ctx.enter_context(tc.tile_pool(name="psS", bufs=3, space="PSUM"))
        psP = ctx.enter_context(tc.tile_pool(name="psP", bufs=2, space="PSUM"))
        psA = ctx.enter_context(tc.tile_pool(name="psA", bufs=1, space="PSUM"))
        psD = ctx.enter_context(tc.tile_pool(name="psD", bufs=1, space="PSUM"))

        # ---------------- load x first (chunked; stats pipeline behind DMA) --
        NC4 = 4                      # head chunks per channel-tile
        CW = N // NC4                # 1024 columns per chunk
        x_d = d["x"]
        x_sb = big.tile([P, CT, N], F32, name="x_sb")
        for t in range(CT):
            for c in range(NC4):
                eng = nc.sync if (t * NC4 + c) % 2 == 0 else nc.gpsimd
                eng.dma_start(out=x_sb[:, t, ds(c * CW, CW)],
                              in_=x_d[ts(t, P), ds(c * CW, CW)])

        # ---------------- constants / weights to SBUF ----------------
        wq_sb = const.tile([P, CT, C], F32R, name="wq_sb")
        wk_sb = const.tile([P, CT, C], F32R, name="wk_sb")
        wv_sb = const.tile([P, CT, C], F32R, name="wv_sb")
        wo_sb = const.tile([P, CT, C], F32R, name="wo_sb")
        for sb, dr in ((wq_sb, d["wq_t"]), (wk_sb, d["wk_t"]),
                       (wv_sb, d["wv_t"]), (wo_sb, d["wo_t"])):
            for ch in range(CT):
                nc.sync.dma_start(out=sb[:, ch, :], in_=dr[ts(ch, P), :])

        bq_sb = const.tile([P, CT], F32, name="bq_sb")
        bk_sb = const.tile([P, CT], F32, name="bk_sb")
        bo_sb = const.tile([P, CT], F32, name="bo_sb")
        gw_sb = const.tile([P, CT], F32, name="gw_sb")
        gb_sb = const.tile([P, CT], F32, name="gb_sb")
        for sb, dr in ((bq_sb, d["b_q"]), (bk_sb, d["b_k"]), (bo_sb, d["b_o"]),
                       (gw_sb, d["gn_w"]), (gb_sb, d["gn_b"])):
            for t in range(CT):
                nc.sync.dma_start(out=sb[:, t:t + 1], in_=dr[t])

        fm_sb = const.tile([P, CT, G], F32, name="fm_sb")
        bm_sb = const.tile([G, CT, P], F32, name="bm_sb")
        for t in range(CT):
            nc.sync.dma_start(out=fm_sb[:, t, :], in_=d["fmask"][t])
            nc.sync.dma_start(out=bm_sb[:, t, :], in_=d["bmask"][t])

        ones_sb = const.tile([P, 1], F32R, name="ones_sb")
        nc.sync.dma_start(out=ones_sb, in_=d["ones_col"])
        zero_sb = const.tile([P, 1], F32, name="zero_sb")
        nc.vector.memset(zero_sb, 0.0)
        eps_sb = const.tile([G, 1], F32, name="eps_sb")
        nc.vector.memset(eps_sb, EPS)

        # ---------------- GroupNorm ----------------
        xn_sb = big.tile([P, CT, N], F32R, name="xn_sb")
        stat = small.tile([P, CT, NC4, 2], F32, name="stat")
        for t in range(CT):
            for c in range(NC4):
                csl = ds(c * CW, CW)
                nc.vector.reduce_sum(out=stat[:, t, c, 0:1], in_=x_sb[:, t, csl],
                                     axis=AX)
                # x^2 into xn (scratch; overwritten below), row-sum into stat
                nc.scalar.activation(out=xn_sb[:, t, csl], in_=x_sb[:, t, csl],
                                     func=AF.Square, bias=zero_sb,
                                     accum_out=stat[:, t, c, 1:2])
                # PE warm-up during the head: slow fp32 matmul on the chunk
                warm = psS.tile([1, NCH], F32, tag="s", name="warm")
                nc.tensor.matmul(warm, lhsT=zero_sb,
                                 rhs=x_sb[:, t, ds(c * CW, NCH)],
                                 start=True, stop=True)

        gps = psS.tile([G, 2], F32, tag="s", name="gps")
        first = True
        for t in range(CT):
            for c in range(NC4):
                nc.tensor.matmul(gps, lhsT=fm_sb[:, t, :], rhs=stat[:, t, c, :],
                                 start=first, stop=(t == CT - 1 and c == NC4 - 1))
                first = False
        grp = small.tile([G, 2], F32, name="grp")    # [mean, rstd]
        gtmp = small.tile([G, 3], F32, name="gtmp")
        nc.vector.tensor_scalar_mul(out=grp[:, 0:1], in0=gps[:, 0:1], scalar1=INV_CNT)
        nc.vector.tensor_scalar_mul(out=gtmp[:, 0:1], in0=gps[:, 1:2], scalar1=INV_CNT)
        nc.vector.tensor_mul(out=gtmp[:, 1:2], in0=grp[:, 0:1], in1=grp[:, 0:1])
        nc.vector.tensor_sub(out=gtmp[:, 2:3], in0=gtmp[:, 0:1], in1=gtmp[:, 1:2])
        nc.scalar.activation(out=gtmp[:, 2:3], in_=gtmp[:, 2:3], func=AF.Sqrt,
                             bias=eps_sb)
        nc.vector.reciprocal(out=grp[:, 1:2], in_=gtmp[:, 2:3])

        ab = small.tile([P, CT, 2], F32, name="ab")  # per-channel scale a, bias b
        for t in range(CT):
            cps = psS.tile([P, 2], F32, tag="s", name="cps")
            nc.tensor.matmul(cps, lhsT=bm_sb[:, t, :], rhs=grp, start=True, stop=True)
            nc.vector.tensor_mul(out=ab[:, t, 0:1], in0=cps[:, 1:2], in1=gw_sb[:, t:t + 1])
            nc.vector.tensor_mul(out=ab[:, t, 1:2], in0=cps[:, 0:1], in1=ab[:, t, 0:1])
            nc.vector.tensor_sub(out=ab[:, t, 1:2], in0=gb_sb[:, t:t + 1], in1=ab[:, t, 1:2])
            nc.vector.tensor_scalar(out=xn_sb[:, t, :], in0=x_sb[:, t, :],
                                    scalar1=ab[:, t, 0:1], scalar2=ab[:, t, 1:2],
                                    op0=OP.mult, op1=OP.add)

        # ---------------- QKV projections ----------------
        q_sb = big.tile([P, CT, N], F32R, name="q_sb")   # (c_half, n)
        k_sb = big.tile([P, CT, N], F32R, name="k_sb")
        vT_sb = big.tile([P, MT, C], F32R, name="vT_sb")  # (n, c), n on partitions

        qki = 0
        for (w_sb, b_sb, o_sb) in ((wq_sb, bq_sb, q_sb), (wk_sb, bk_sb, k_sb)):
            for tq in range(CT):
                for nch in range(NNCH):
                    ps = psS.tile([P, NCH], F32, tag="s", name="psqk")
                    for ch in range(CT):
                        nc.tensor.matmul(
                            ps,
                            lhsT=w_sb[:, ch, ts(tq, P)],
                            rhs=xn_sb[:, ch, ds(nch * NCH, NCH)],
                            start=(ch == 0), stop=(ch == CT - 1))
                    osl = o_sb[:, tq, ds(nch * NCH, NCH)]
                    if qki % 2 == 0:
                        nc.vector.tensor_scalar_add(out=osl, in0=ps,
                                                    scalar1=b_sb[:, tq:tq + 1])
                    else:
                        nc.scalar.activation(out=osl, in_=ps, func=AF.Identity,
                                             bias=b_sb[:, tq:tq + 1])
                    qki += 1

        for mt in range(MT):
            ps = psS.tile([P, C], F32, tag="s", name="psv")
            for ch in range(CT):
                nc.tensor.matmul(ps,
                                 lhsT=xn_sb[:, ch, ts(mt, P)],
                                 rhs=wv_sb[:, ch, :],
                                 start=(ch == 0), stop=(ch == CT - 1))
            # b_v is folded into b_out host-side (softmax rows sum to 1)
            if mt % 2 == 0:
                nc.vector.tensor_copy(out=vT_sb[:, mt, :], in_=ps)
            else:
                nc.scalar.copy(out=vT_sb[:, mt, :], in_=ps)

        # ---------------- attention + output projection ----------------
        for nch in range(NNCH):
            nsl = ds(nch * NCH, NCH)
            attn = psA.tile([P, CT, NCH], F32, tag="attn", name="attn")
            den = psD.tile([1, NCH], F32, tag="den", name="den")
            prev_e = None
            for mt in range(MT):
                s = psS.tile([P, NCH], F32, tag="s", name="s")
                for ch in range(CT):
                    nc.tensor.matmul(s,
                                     lhsT=k_sb[:, ch, ts(mt, P)],
                                     rhs=q_sb[:, ch, nsl],
                                     start=(ch == 0), stop=(ch == CT - 1))
                e = work.tile([P, NCH], F32R, tag="e", name="e", bufs=4)
                nc.scalar.activation(out=e, in_=s, func=AF.Exp, bias=zero_sb,
                                     scale=SCALE)
                for ch in range(CT):
                    nc.tensor.matmul(attn[:, ch, :],
                                     lhsT=vT_sb[:, mt, ts(ch, P)],
                                     rhs=e,
                                     start=(mt == 0), stop=(mt == MT - 1))
                nc.tensor.matmul(den, lhsT=ones_sb, rhs=e,
                                 start=(mt == 0), stop=(mt == MT - 1))

            den_sb = small.tile([1, NCH], F32, tag="den_sb", name="den_sb", bufs=2)
            nc.vector.tensor_copy(out=den_sb, in_=den)
            rden = small.tile([1, NCH], F32, tag="rden", name="rden", bufs=2)
            rscr = small.tile([1, NCH], F32, tag="rscr", name="rscr", bufs=2)
            nc.vector.reciprocal_approx_accurate(out=rden, in_=den_sb, scratch=rscr)
            rdenb = outp.tile([P, NCH], F32, tag="rdenb", name="rdenb", bufs=2)
            nc.gpsimd.partition_broadcast(rdenb, rden)

            atts = []
            for ch in range(CT):
                att = outp.tile([P, NCH], F32R, tag="att", name=f"att{ch}", bufs=3)
                nc.vector.tensor_copy(out=att, in_=attn[:, ch, :])
                atts.append(att)

            for co in range(CT):
                pj = psP.tile([P, NCH], F32, tag="proj", name="pj")
                for ch in range(CT):
                    nc.tensor.matmul(pj,
                                     lhsT=wo_sb[:, ch, ts(co, P)],
                                     rhs=atts[ch],
                                     start=(ch == 0), stop=(ch == CT - 1))
                f = outp.tile([P, NCH], F32, tag="fout", name="f", bufs=2)
                nc.vector.tensor_tensor(out=f, in0=pj, in1=rdenb, op=OP.mult)
                nc.vector.scalar_tensor_tensor(out=f, in0=f, scalar=bo_sb[:, co:co + 1],
                                               in1=x_sb[:, co, nsl],
                                               op0=OP.add, op1=OP.add)
                nc.sync.dma_start(out=out_d[ts(co, P), nsl], in_=f)


def build_program():
    nc = bacc.Bacc("TRN2", target_bir_lowering=False, debug=False, num_devices=B)
    d = {}

    def din(name, shape, dt_=F32):
        d[name] = nc.dram_tensor(name, list(shape), dt_, kind="ExternalInput").ap()

    din("x", (C, N))
    din("wq_t", (C, C), F32R)
    din("wk_t", (C, C), F32R)
    din("wv_t", (C, C), F32R)
    din("wo_t", (C, C), F32R)
    din("b_q", (CT, P, 1))
    din("b_k", (CT, P, 1))
    din("b_o", (CT, P, 1))
    din("gn_w", (CT, P, 1))
    din("gn_b", (CT, P, 1))
    din("fmask", (CT, P, G))
    din("bmask", (CT, G, P))
    din("ones_col", (P, 1), F32R)
    out_d = nc.dram_tensor("out", [C, N], F32, kind="ExternalOutput").ap()

    with tile.TileContext(nc) as tc:
        _emit(tc, d, out_d)
    nc.compile()
    return nc


_PROG = None


def _get_program():
    global _PROG
    if _PROG is None:
        _PROG = build_program()
    return _PROG


def make_in_maps(inputs):
    x = np.ascontiguousarray(np.asarray(inputs["x"], dtype=np.float32))
    w_qkv = np.asarray(inputs["w_qkv"], dtype=np.float32)
    b_qkv = np.asarray(inputs["b_qkv"], dtype=np.float32)
    w_out = np.asarray(inputs["w_out"], dtype=np.float32)
    b_out = np.asarray(inputs["b_out"], dtype=np.float32)
    gn_scale = np.asarray(inputs["gn_scale"], dtype=np.float32)
    gn_bias = np.asarray(inputs["gn_bias"], dtype=np.float32)

    fmask = np.zeros((CT, P, G), dtype=np.float32)
    for t in range(CT):
        for p in range(P):
            fmask[t, p, (t * P + p) // GSZ] = 1.0
    bmask = np.ascontiguousarray(fmask.transpose(0, 2, 1))

    common = {
        "wq_t": np.ascontiguousarray(w_qkv[0:C].T),
        "wk_t": np.ascontiguousarray(w_qkv[C:2 * C].T),
        "wv_t": np.ascontiguousarray(w_qkv[2 * C:3 * C].T),
        "wo_t": np.ascontiguousarray(w_out.T),
        "b_q": np.ascontiguousarray(b_qkv[0:C].reshape(CT, P, 1)),
        "b_k": np.ascontiguousarray(b_qkv[C:2 * C].reshape(CT, P, 1)),
        "b_o": np.ascontiguousarray((b_out + w_out @ b_qkv[2 * C:3 * C]).reshape(CT, P, 1)),
        "gn_w": np.ascontiguousarray(gn_scale.reshape(CT, P, 1)),
        "gn_b": np.ascontiguousarray(gn_bias.reshape(CT, P, 1)),
        "fmask": fmask,
        "bmask": bmask,
        "ones_col": np.ones((P, 1), dtype=np.float32),
    }
    return [dict(common, x=np.ascontiguousarray(x[b].reshape(C, N)))
            for b in range(B)]


def run(inputs, trace=False):
    nc = _get_program()
    in_maps = make_in_maps(inputs)
    res = bass_utils.run_bass_kernel_spmd(nc, in_maps, core_ids=list(range(B)),
                                          trace=trace)
    out = np.stack([res.results[b]["out"] for b in range(B)])
    return out.reshape(B, C, HH, WW), res


def kernel(**inputs):
    out, _ = run(inputs, trace=False)
    return out



# revision 14
# speedup vs baseline: 1.6720x; 1.6720x over previous
"""AttentionBlock (GroupNorm + single-head NxN attention + residual) on 8 TRN2 cores.

Data-parallel: batch dim (B=8) sharded 1 image per NeuronCore. Each core runs
the full block for its image:

  x (C=256, N=4096) -> GroupNorm(8 groups) -> q,k = Wq,Wk @ xn (fp8)
  u = (W_out @ W_v) @ xn (fp8, output projection folded into V)
  s = k^T q (fp8 DoubleRow matmul, contraction C=256 in one pass)
  e = exp(s/16 - 2) (scalar engine, fp8 out; the -2 shift cancels in softmax)
  attn_u = u @ e, den = ones @ e (fp8 DoubleRow)
  out = attn_u * (1/den) + b_out' + x

All heavy matmuls run as fp8e4m3 with MatmulPerfMode.DoubleRow (K=256 per
matmul). The softmax is unnormalized; 1/den commutes through the (folded)
output projection and is applied once at the end. fp8 noise only touches the
attention branch, which is small versus the fp32 residual, keeping max-rel
error ~1e-3.
"""

import sys

if "/opt/trn_rl_repo" not in sys.path:
    sys.path.insert(0, "/opt/trn_rl_repo")

import numpy as np
import ml_dtypes

import concourse.bass as bass
import concourse.bacc as bacc
import concourse.tile as tile
import concourse.mybir as mybir
from concourse import bass_utils

# Problem dims (hardcoded per spec)
B, C, HH, WW = 8, 256, 64, 64
N = HH * WW            # 4096
G = 8                  # groupnorm groups
GSZ = C // G           # 32 channels/group
EPS = 1e-5
P = 128                # SBUF partitions
CT = C // P            # 2 channel tiles (also the DoubleRow K-tile count)
NCH = 512              # query-chunk width (free dim per matmul)
NNCH = N // NCH        # 8
MT = N // P            # 32 key tiles of 128
JT = MT // 2           # 16 key supertiles of 256 (DoubleRow)
SCALE = 1.0 / np.sqrt(C)
SHIFT = 4.0            # exp(s*SCALE - SHIFT); cancels in softmax, keeps e well
                       # inside fp8e4m3 range (max observed s*SCALE is ~8 ->
                       # e^4 = 55 << 240; overflow headroom up to s*SCALE=9.4)
INV_CNT = 1.0 / (GSZ * N)

F32 = mybir.dt.float32
F32R = mybir.dt.float32r
F8 = mybir.dt.float8e4
DR = mybir.MatmulPerfMode.DoubleRow
NP_F8 = ml_dtypes.float8_e4m3


def _emit(tc, d, out_d):
    from contextlib import ExitStack

    nc = tc.nc
    AF = mybir.ActivationFunctionType
    OP = mybir.AluOpType
    AX = mybir.AxisListType.X
    ts, ds = bass.ts, bass.ds

    with ExitStack() as ctx:
        const = ctx.enter_context(tc.tile_pool(name="const", bufs=1))
        big = ctx.enter_context(tc.tile_pool(name="big", bufs=1))
        work = ctx.enter_context(tc.tile_pool(name="work", bufs=4))
        small = ctx.enter_context(tc.tile_pool(name="small", bufs=4))
        outp = ctx.enter_context(tc.tile_pool(name="outp", bufs=3))

        # ---------------- load x (chunked; stats pipeline behind DMA) --------
        NC4 = 4                      # chunks per channel-tile
        CW = N // NC4                # 1024 columns per chunk
        x_d = d["x"]
        x_sb = big.tile([P, CT, N], F32, name="x_sb")
        dma_engs = (nc.sync, nc.gpsimd, nc.scalar)
        for t in range(CT):
            for c in range(NC4):
                eng = dma_engs[(t * NC4 + c) % len(dma_engs)]
                eng.dma_start(out=x_sb[:, t, ds(c * CW, CW)],
                              in_=x_d[ts(t, P), ds(c * CW, CW)])

        # ---------------- constants / weights to SBUF ----------------
        wq_sb = const.tile([P, CT, C], F8, name="wq_sb")
        wk_sb = const.tile([P, CT, C], F8, name="wk_sb")
        wov_sb = const.tile([P, CT, C], F8, name="wov_sb")
        nc.sync.dma_start(out=wq_sb, in_=d["wq_p"])
        nc.gpsimd.dma_start(out=wk_sb, in_=d["wk_p"])
        nc.scalar.dma_start(out=wov_sb, in_=d["wov_p"])

        # bias5 rows: 0=b_q 1=b_k 2=b_o 3=gn_w 4=gn_b; SBUF [P, 5, CT]
        b5_sb = const.tile([P, 5, CT], F32, name="b5_sb")
        nc.sync.dma_start(out=b5_sb, in_=d["bias5"])
        bq_sb = b5_sb[:, 0, :]
        bk_sb = b5_sb[:, 1, :]
        bo_sb = b5_sb[:, 2, :]
        gw_sb = b5_sb[:, 3, :]
        gb_sb = b5_sb[:, 4, :]

        fm_sb = const.tile([P, CT, G], F32, name="fm_sb")
        bm_sb = const.tile([G, CT, P], F32, name="bm_sb")
        for t in range(CT):
            nc.gpsimd.dma_start(out=fm_sb[:, t, :], in_=d["fmask"][t])
            nc.gpsimd.dma_start(out=bm_sb[:, t, :], in_=d["bmask"][t])

        ones_sb = const.tile([P, CT, P], F8, name="ones_sb")
        nc.vector.memset(ones_sb, 1.0)
        zero_sb = const.tile([P, 1], F32, name="zero_sb")
        nc.vector.memset(zero_sb, 0.0)
        nshift_sb = const.tile([P, 1], F32, name="nshift_sb")
        nc.vector.memset(nshift_sb, -SHIFT)
        eps_sb = const.tile([G, 1], F32, name="eps_sb")
        nc.vector.memset(eps_sb, EPS)

        # ---------------- GroupNorm stats (early PSUM pool, released) -------
        sqscr = small.tile([P, CW], F32, name="sqscr", bufs=2)
        stat = small.tile([P, CT, NC4, 2], F32, name="stat")
        ab = small.tile([P, CT, 2], F32, name="ab")  # per-channel scale, bias
        with tc.tile_pool(name="psI", bufs=1, space="PSUM") as psI:
            for t in range(CT):
                for c in range(NC4):
                    csl = ds(c * CW, CW)
                    nc.vector.reduce_sum(out=stat[:, t, c, 0:1],
                                         in_=x_sb[:, t, csl], axis=AX)
                    nc.scalar.activation(out=sqscr, in_=x_sb[:, t, csl],
                                         func=AF.Square, bias=zero_sb,
                                         accum_out=stat[:, t, c, 1:2])
                    # PE warm-up during the head: slow fp32 matmul on the chunk
                    warm = psI.tile([1, NCH], F32, tag="warm", name="warm")
                    nc.tensor.matmul(warm, lhsT=zero_sb,
                                     rhs=x_sb[:, t, ds(c * CW, NCH)],
                                     start=True, stop=True)

            gps = psI.tile([G, 2], F32, tag="gps", name="gps")
            first = True
            for t in range(CT):
                for c in range(NC4):
                    nc.tensor.matmul(gps, lhsT=fm_sb[:, t, :],
                                     rhs=stat[:, t, c, :],
                                     start=first,
                                     stop=(t == CT - 1 and c == NC4 - 1))
                    first = False
            grp = small.tile([G, 2], F32, name="grp")    # [mean, rstd]
            gtmp = small.tile([G, 3], F32, name="gtmp")
            nc.vector.tensor_scalar_mul(out=grp[:, 0:1], in0=gps[:, 0:1],
                                        scalar1=INV_CNT)
            nc.vector.tensor_scalar_mul(out=gtmp[:, 0:1], in0=gps[:, 1:2],
                                        scalar1=INV_CNT)
            nc.vector.tensor_mul(out=gtmp[:, 1:2], in0=grp[:, 0:1], in1=grp[:, 0:1])
            nc.vector.tensor_sub(out=gtmp[:, 2:3], in0=gtmp[:, 0:1], in1=gtmp[:, 1:2])
            nc.scalar.activation(out=gtmp[:, 2:3], in_=gtmp[:, 2:3], func=AF.Sqrt,
                                 bias=eps_sb)
            nc.vector.reciprocal(out=grp[:, 1:2], in_=gtmp[:, 2:3])

            for t in range(CT):
                cps = psI.tile([P, 2], F32, tag="cps", name="cps")
                nc.tensor.matmul(cps, lhsT=bm_sb[:, t, :], rhs=grp,
                                 start=True, stop=True)
                nc.vector.tensor_mul(out=ab[:, t, 0:1], in0=cps[:, 1:2],
                                     in1=gw_sb[:, t:t + 1])
                nc.vector.tensor_mul(out=ab[:, t, 1:2], in0=cps[:, 0:1],
                                     in1=ab[:, t, 0:1])
                nc.vector.tensor_sub(out=ab[:, t, 1:2], in0=gb_sb[:, t:t + 1],
                                     in1=ab[:, t, 1:2])

        # ---------------- main PSUM pools (after psI released) ----------------
        # 4 banks scores pairs + 2 banks attn + 2 banks den = 8
        psS = ctx.enter_context(tc.tile_pool(name="psS", bufs=2, space="PSUM"))
        psA = ctx.enter_context(tc.tile_pool(name="psA", bufs=1, space="PSUM"))
        psD = ctx.enter_context(tc.tile_pool(name="psD", bufs=2, space="PSUM"))

        # ---------------- xn = a*x + b in fp8 (vector + gpsimd) -------------
        xn_sb = big.tile([P, CT, N], F8, name="xn_sb")
        for t in range(CT):
            eng = nc.vector if t == 0 else nc.gpsimd
            eng.tensor_scalar(out=xn_sb[:, t, :], in0=x_sb[:, t, :],
                              scalar1=ab[:, t, 0:1], scalar2=ab[:, t, 1:2],
                              op0=OP.mult, op1=OP.add)

        q_sb = big.tile([P, CT, N], F8, name="q_sb")   # (c, n)
        k_sb = big.tile([P, CT, N], F8, name="k_sb")
        u_sb = big.tile([P, JT, CT, C], F8, name="u_sb")  # uT packed (m, c)

        def emit_q(nch):
            """q chunk pair (both tq) for one nch; copies on vector."""
            nsl = ds(nch * NCH, NCH)
            pr = psS.tile([P, CT, NCH], F32, tag="s", name="psq")
            for tq in range(CT):
                nc.tensor.matmul(pr[:, tq, :], lhsT=wq_sb[:, :, ts(tq, P)],
                                 rhs=xn_sb[:, :, nsl],
                                 start=True, stop=True, perf_mode=DR)
            for tq in range(CT):
                nc.vector.tensor_scalar_add(out=q_sb[:, tq, nsl],
                                            in0=pr[:, tq, :],
                                            scalar1=bq_sb[:, tq:tq + 1])

        def emit_k(c, eng_idx):
            """k chunk pair (both tq) for nch_k c; copies on scalar/vector."""
            nsl = ds(c * NCH, NCH)
            pr = psS.tile([P, CT, NCH], F32, tag="s", name="psk")
            for tq in range(CT):
                nc.tensor.matmul(pr[:, tq, :], lhsT=wk_sb[:, :, ts(tq, P)],
                                 rhs=xn_sb[:, :, nsl],
                                 start=True, stop=True, perf_mode=DR)
            for tq in range(CT):
                if eng_idx == 0:
                    nc.scalar.activation(out=k_sb[:, tq, nsl], in_=pr[:, tq, :],
                                         func=AF.Identity,
                                         bias=bk_sb[:, tq:tq + 1])
                else:
                    nc.vector.tensor_scalar_add(out=k_sb[:, tq, nsl],
                                                in0=pr[:, tq, :],
                                                scalar1=bk_sb[:, tq:tq + 1])

        def emit_u(g, eng_idx):
            """u supertiles 2g, 2g+1 (mt 4g..4g+3); one whole-pair copy."""
            pr = psS.tile([P, CT, NCH], F32, tag="s", name="psu")
            for h in range(4):
                mt = 4 * g + h
                nc.tensor.matmul(pr[:, h // 2, ds((h % 2) * C, C)],
                                 lhsT=xn_sb[:, :, ts(mt, P)],
                                 rhs=wov_sb, start=True, stop=True, perf_mode=DR)
            dst = u_sb[:, 2 * g:2 * g + 2, :, :]
            if eng_idx == 0:
                nc.scalar.copy(out=dst, in_=pr)
            else:
                nc.vector.tensor_copy(out=dst, in_=pr)

        # ---------------- pre-stage: q(nch0), all k, all u -------------------
        emit_q(0)
        for c in range(NNCH):
            emit_k(c, c % 2)
            emit_u(c, (c + 1) % 2)

        # ---------------- attention + fused output projection ----------------
        # Software pipeline over all (nch, j): scores for step idx+1 are
        # emitted before attn/den of step idx so the PE keeps the scalar
        # engine's exp stream fed.
        steps = [(nch, j) for nch in range(NNCH) for j in range(JT)]

        def emit_scores(nch, j):
            nsl = ds(nch * NCH, NCH)
            pr = psS.tile([P, CT, NCH], F32, tag="s", name="pss")
            for i in range(2):
                mt = 2 * j + i
                nc.tensor.matmul(pr[:, i, :], lhsT=k_sb[:, :, ts(mt, P)],
                                 rhs=q_sb[:, :, nsl],
                                 start=True, stop=True, perf_mode=DR)
            return pr

        attn = None
        den = None
        pr_cur = emit_scores(0, 0)
        for idx, (nch, j) in enumerate(steps):
            nsl = ds(nch * NCH, NCH)
            if j == 0:
                attn = psA.tile([P, CT, NCH], F32, tag="attn", name="attn")
                den = psD.tile([P, NCH], F32, tag="den", name="den")
            e = work.tile([P, CT, NCH], F8, tag="e", name="e", bufs=4)
            nc.scalar.activation(out=e, in_=pr_cur, func=AF.Exp,
                                 bias=nshift_sb, scale=SCALE)
            if idx + 1 < len(steps):
                pr_cur = emit_scores(*steps[idx + 1])
            if j == 2 and nch + 1 < NNCH:
                emit_q(nch + 1)
            for co in range(CT):
                nc.tensor.matmul(attn[:, co, :],
                                 lhsT=u_sb[:, j, :, ts(co, P)],
                                 rhs=e,
                                 start=(j == 0), stop=(j == JT - 1),
                                 perf_mode=DR)
            nc.tensor.matmul(den, lhsT=ones_sb, rhs=e,
                             start=(j == 0), stop=(j == JT - 1),
                             perf_mode=DR)
            if j != JT - 1:
                continue

            # -------- finalize this nch --------
            rden = outp.tile([P, NCH], F32, tag="rden", name="rden", bufs=2)
            rscr = outp.tile([P, NCH], F32, tag="rscr", name="rscr", bufs=2)
            nc.vector.reciprocal_approx_accurate(out=rden, in_=den, scratch=rscr)
            for co in range(CT):
                f = outp.tile([P, NCH], F32, tag="fout", name="f", bufs=3)
                nc.vector.tensor_tensor(out=f, in0=attn[:, co, :], in1=rden,
                                        op=OP.mult)
                nc.vector.scalar_tensor_tensor(out=f, in0=f,
                                               scalar=bo_sb[:, co:co + 1],
                                               in1=x_sb[:, co, nsl],
                                               op0=OP.add, op1=OP.add)
                nc.sync.dma_start(out=out_d[ts(co, P), nsl], in_=f)


def build_program():
    nc = bacc.Bacc("TRN2", target_bir_lowering=False, debug=False, num_devices=B)
    d = {}

    def din(name, shape, dt_=F32):
        d[name] = nc.dram_tensor(name, list(shape), dt_, kind="ExternalInput").ap()

    din("x", (C, N))
    din("wq_p", (P, CT, C), F8)
    din("wk_p", (P, CT, C), F8)
    din("wov_p", (P, CT, C), F8)
    din("bias5", (P, 5, CT))
    din("fmask", (CT, P, G))
    din("bmask", (CT, G, P))
    out_d = nc.dram_tensor("out", [C, N], F32, kind="ExternalOutput").ap()

    with tile.TileContext(nc) as tc:
        _emit(tc, d, out_d)
    nc.compile()
    return nc


_PROG = None


def _get_program():
    global _PROG
    if _PROG is None:
        _PROG = build_program()
    return _PROG


def _pack_w(w):
    """[c_out, c_in] fp32 -> [p, r, c_out] fp8 with c_in = r*128 + p."""
    wt = np.ascontiguousarray(w.T)                   # [c_in, c_out]
    return np.ascontiguousarray(
        wt.reshape(CT, P, C).transpose(1, 0, 2)).astype(NP_F8)


def make_in_maps(inputs):
    x = np.ascontiguousarray(np.asarray(inputs["x"], dtype=np.float32))
    w_qkv = np.asarray(inputs["w_qkv"], dtype=np.float32)
    b_qkv = np.asarray(inputs["b_qkv"], dtype=np.float32)
    w_out = np.asarray(inputs["w_out"], dtype=np.float32)
    b_out = np.asarray(inputs["b_out"], dtype=np.float32)
    gn_scale = np.asarray(inputs["gn_scale"], dtype=np.float32)
    gn_bias = np.asarray(inputs["gn_bias"], dtype=np.float32)

    fmask = np.zeros((CT, P, G), dtype=np.float32)
    for t in range(CT):
        for p in range(P):
            fmask[t, p, (t * P + p) // GSZ] = 1.0
    bmask = np.ascontiguousarray(fmask.transpose(0, 2, 1))

    w_q = w_qkv[0:C]
    w_k = w_qkv[C:2 * C]
    w_v = w_qkv[2 * C:3 * C]
    w_ov = w_out @ w_v                                # folded output projection

    bias5 = np.stack([
        b_qkv[0:C],                            # b_q
        b_qkv[C:2 * C],                        # b_k
        b_out + w_out @ b_qkv[2 * C:3 * C],    # b_o (with folded b_v)
        gn_scale,
        gn_bias,
    ])                                          # [5, C]
    common = {
        "wq_p": _pack_w(w_q),
        "wk_p": _pack_w(w_k),
        "wov_p": _pack_w(w_ov),
        "bias5": np.ascontiguousarray(
            bias5.reshape(5, CT, P).transpose(2, 0, 1)),
        "fmask": fmask,
        "bmask": bmask,
    }
    return [dict(common, x=np.ascontiguousarray(x[b].reshape(C, N)))
            for b in range(B)]


def run(inputs, trace=False):
    nc = _get_program()
    in_maps = make_in_maps(inputs)
    res = bass_utils.run_bass_kernel_spmd(nc, in_maps, core_ids=list(range(B)),
                                          trace=trace)
    out = np.stack([res.results[b]["out"] for b in range(B)])
    return out.reshape(B, C, HH, WW), res


def kernel(**inputs):
    out, _ = run(inputs, trace=False)
    return out
